# revision 4
# baseline (speedup 1.0000x reference)
"""Trainium2 Bass kernel for causal attention with relative-position bias.

Problem (hardcoded): B=16 heads, S=2048, Dh=64, fp32 I/O.
  dots = Q@K^T; bias pos=Q@R_w^T+R_b gathered by sign(j-i)+1; causal mask
  (-1e10 above diag); softmax(dots/sqrt(512)); out = probs@V.

Algebra: within row q the gathered bias is a constant pos0[q] for k<q and
pos1[q] at k==q (k>q masked). Softmax is invariant to per-row constants, so
only the diagonal needs the delta v[q] = Q[q].(R_w[1]-R_w[0]) + R_b[1]-R_b[0].
Logits are small (|z|<=~2.2) so exp runs without max subtraction.

Layout: scores computed transposed, S^T[k,q] (k on partitions):
  S^T = (K^T chunk).T @ Q^T      (lhsT=K^T[64,128], rhs=Q^T[64,ncols])
  out^T[d,q]+denominator row = [V|1].T @ exp(S^T)  (accumulated over chunks)

Diagonal tiles: one accumulate-matmul  A_ui^T @ dcomb_ki  follows the QK
matmul, where A_ui[m,k]=[m<=k] (so the product is a column cumsum) and
dcomb[m,q] = v[q]*([m==q]-[m==q+1]) + (-V0*[m>q] + rbd*[m==q]).  The cumsum
turns this into v[q]*[k==q] - V0*(k-q)*[k>q] + rbd*[k>=q]: position-bias
delta on the diagonal, -V0*(k-q) above it (exp == 0 in fp16), nothing below.

Perf structure (v2):
 - K/Q loaded fp32 on the sync HWDGE ring, cast fp16 on DVE; V loaded via
   gpsimd SWDGE cast-DMA straight into the fp16 [V|1] strip.  The scalar
   (ACT) engine runs ONLY exp -- it is the critical resource (~1 elem/cyc
   @1.2GHz over the whole causal region).
 - Q^T/K^T via xbar-DMA transposes (fold on sync, unfold on gpsimd); the
   transposed [64,S] strips are duplicated to partitions 64..127 so QK
   matmul chunks run PAIRED in independent PE row-groups (contraction is
   only 64): two 512-col QK chunks execute concurrently.
 - exp is batched: score fills are grouped (pair of fills in adjacent PSUM
   slots -> one wide ACTIVATE), scheduled so consecutive groups never share
   PSUM slots (QK of group g+1 overlaps exp of group g).
 - Phases run [1, 0] per head: phase 1's first 8 fills are full-width and
   non-diagonal, so the exp stream starts without waiting for the dcomb
   (diagonal) DVE chain.
 - Output epilogue (transpose-back, divide, store) runs per 512-column
   block as soon as that block's PV accumulation stops.

Sharding: 16 heads -> 8 NeuronCores, 2 heads/core, no communication.
"""

import os
import sys

if "/opt/trn_rl_repo" not in sys.path:
    sys.path.insert(0, "/opt/trn_rl_repo")

import numpy as np

import concourse.bacc as bacc
import concourse.mybir as mybir
import concourse.tile as tile
from concourse.bass_utils import run_bass_kernel_spmd
from concourse.masks import make_identity, make_lower_triangular, make_upper_triangular

B, S, DH = 16, 2048, 64
N_CORES = 8
HPC = B // N_CORES  # heads per core
P = 128
NT = S // P  # 16 q/k tiles per head
HT = NT // 2  # tiles per load half
HW = HT * DH  # elements per half (free dim)
VW = 66  # V row width in SBUF: 64 values + ones col + pad (66*2B keeps 4B align)
OW = 80  # out^T rows padded to xbar multiple of 16 (64 vals + denom + 15 pad)
PH = 1024  # q-phase width
INV_SCALE = float(1.0 / np.sqrt(np.float32(512.0)))
V0 = 1000.0  # per-step causal mask magnitude; exp((z-V0)/scale) == 0 in fp16

f16 = mybir.dt.float16
f32 = mybir.dt.float32


class _Fill:
    __slots__ = ("ki", "q0", "base", "n", "diag")

    def __init__(self, ki, q0, base, n, diag):
        self.ki, self.q0, self.base, self.n, self.diag = ki, q0, base, n, diag


def _plan_phase(lo, hi):
    """Group the phase's fills into ACT groups: pairs (f0 width in {512,1024}
    so the pair region is gap-free) and singles, interleaved so consecutive
    groups can take disjoint PSUM slots."""
    fills = []
    for ki in range(NT):
        q0 = P * ki
        base = max(q0, lo)
        if base < hi:
            fills.append(_Fill(ki, q0, base, hi - base, base == q0))
    avail = sorted(fills, key=lambda f: -f.n)
    used = set()
    pairs = []
    for f0 in avail:
        if id(f0) in used or f0.n not in (PH, 512):
            continue
        f1 = next((g for g in avail if id(g) not in used and g is not f0), None)
        if f1 is None:
            break
        used.add(id(f0))
        used.add(id(f1))
        pairs.append((f0, f1))
    singles = [f for f in avail if id(f) not in used]
    # alternation needs len(singles) >= len(pairs) - 1; demote smallest pairs
    while len(pairs) > len(singles) + 1:
        f0, f1 = pairs.pop()
        singles += [f0, f1]
    singles.sort(key=lambda f: -f.n)
    seq = []  # list of [(fill, off), ...]
    pi = si = 0
    while pi < len(pairs) or si < len(singles):
        if pi < len(pairs):
            f0, f1 = pairs[pi]
            seq.append([(f0, 0), (f1, f0.n)])
            pi += 1
        if si < len(singles):
            seq.append([(singles[si], 0)])
            si += 1
    return seq


def _assign_slots(seq, slot_state):
    """Greedy PSUM slot assignment (3 slots of 1024 cols).  Pairs use slots
    (0,1) or (1,2); singles use one slot.  Consecutive groups (including
    across phase/head boundaries, threaded via slot_state) get disjoint
    slots so QK(g+1) overlaps exp(g)."""
    out = []
    for items in seq:
        width = items[-1][1] + items[-1][0].n
        if width > PH:  # pair spanning two slots
            options = [({0, 1}, 0), ({1, 2}, PH)]
        else:
            options = [({2}, 2 * PH), ({0}, 0), ({1}, PH)]
        for slots, gcol in options:
            if not (slots & slot_state["prev"]):
                break
        else:
            raise AssertionError("no disjoint slot assignment")
        slot_state["prev"] = slots
        out.append((gcol, items))
    return out


def _emit(ctx, tc, q_d, k_d, v_d, rw_d, rb_d, out_d):
    nc = tc.nc
    AF = mybir.ActivationFunctionType

    const = ctx.enter_context(tc.tile_pool(name="const", bufs=1))
    ld = ctx.enter_context(tc.tile_pool(name="ld", bufs=3))
    foldp = ctx.enter_context(tc.tile_pool(name="foldp", bufs=3))
    hp = ctx.enter_context(tc.tile_pool(name="hp", bufs=2))
    slabp = ctx.enter_context(tc.tile_pool(name="slab", bufs=3))
    outp = ctx.enter_context(tc.tile_pool(name="outp", bufs=2))
    pscp = ctx.enter_context(tc.tile_pool(name="psc", bufs=1, space="PSUM"))
    pout = ctx.enter_context(tc.tile_pool(name="pout", bufs=1, space="PSUM"))

    # gpsimd: junk buffer + mask constants (all needed before the dcomb chain)
    junk = const.tile([P, 512], f16)
    nc.gpsimd.memset(junk[:], 0.0)
    aui = const.tile([P, P], f16)  # A_ui[m,k] = 1 for m<=k (cumsum matmul lhsT)
    make_upper_triangular(nc, aui[:], val=1.0, diag=True)
    # idp[m,j] = [m==j] - [m==j+1]: cumsum of idp*v gives diag(v)
    idp = const.tile([P, P], f16)
    make_identity(nc, idp[:])
    nc.gpsimd.affine_select(
        out=idp[:], in_=idp[:], compare_op=mybir.AluOpType.not_equal,
        fill=-1.0, base=-1, pattern=[[-1, P]], channel_multiplier=1,
    )
    id01 = const.tile([P, P], mybir.dt.int8)
    make_identity(nc, id01[:])
    # bneg[m,q] = -V0*[m>q] + rbd*[m==q]; cumsum gives the causal mask + rbd
    bneg = const.tile([P, P], f16)
    make_lower_triangular(nc, bneg[:], val=-V0, diag=False)

    # sync: broadcast R_w rows 0+1 and R_b[0:2] to all partitions
    rbc = const.tile([P, 2 * DH + 2], f32)
    nc.sync.dma_start(
        out=rbc[:, 0 : 2 * DH], in_=rw_d[0:2, :].flatten()[None, :].partition_broadcast(P)
    )
    nc.sync.dma_start(
        out=rbc[:, 2 * DH : 2 * DH + 2], in_=rb_d[None, 0:2].partition_broadcast(P)
    )

    # PSUM: one 6-bank score region (3 slots of 1024 fp32) + 2-bank outT
    psc = pscp.tile([P, 3 * PH], f32)

    # preload the exp table set at t=0 so ACT_TABLE_LOAD overlaps input DMA
    tl16 = const.tile([P, 8], f16)
    nc.scalar.activation(tl16[:], junk[:, 0:8], AF.Exp, scale=0.0)

    # PE warm-up: junk matmuls while DMAs load (HAM at 8/8 by the first QK)
    for _ in range(6):
        nc.tensor.matmul(
            psc[:, 0:512], lhsT=junk[:, 0:P], rhs=junk[:], start=True,
            stop=True, skip_group_check=True,
        )

    # DVE: rbc-dependent constants (rd16 = R_w[1]-R_w[0]; rbd onto bneg diag)
    rd16 = const.tile([P, DH], f16)
    rbraw = const.tile([P, 2], f32)
    rbf16 = const.tile([P, 1], f16)
    nc.vector.tensor_sub(rd16[:], rbc[:, DH : 2 * DH], rbc[:, 0:DH])
    nc.vector.tensor_sub(
        rbraw[:, 0:1], rbc[:, 2 * DH + 1 : 2 * DH + 2], rbc[:, 2 * DH : 2 * DH + 1]
    )
    nc.vector.tensor_copy(rbf16[:], rbraw[:, 0:1])
    nc.vector.copy_predicated(bneg[:], id01[:], rbf16[:, 0:1].to_broadcast([P, P]))

    # ---- per-head tiles -------------------------------------------------
    def head_tiles():
        t = {}
        t["qf"] = hp.tile([P, NT * DH], f16, tag="qf", name="qf")
        t["kf"] = hp.tile([P, NT * DH], f16, tag="kf", name="kf")
        t["ktall"] = hp.tile([P, S], f16, tag="ktall", name="ktall")  # rows 0:64 K^T, 64:128 copy
        t["qtall"] = hp.tile([P, S], f16, tag="qtall", name="qtall")
        t["vaug"] = hp.tile([P, NT * VW], f16, tag="vaug", name="vaug")
        t["dcomb"] = hp.tile([P, NT * P], f16, tag="dcomb", name="dcomb")
        t["vq"] = hp.tile([P, NT], f32, tag="vq", name="vq")
        t["vq16"] = hp.tile([P, NT], f16, tag="vq16", name="vq16")
        t["t2"] = hp.tile([P, NT * DH], f16, tag="t2", name="t2")
        t["outTs"] = outp.tile([OW, S], f16, tag="outTs", name="outTs")
        return t

    # ---- emission helpers (each touches one engine queue) ---------------
    def load_half(src_d, h, hf, tag):  # sync ring, fp32
        dst = ld.tile([P, HW], f32, tag=tag)
        nc.sync.dma_start(
            out=dst[:].rearrange("p (n d) -> p n d", d=DH),
            in_=src_d[h].rearrange("(n p) d -> p n d", p=P)[
                :, hf * HT : (hf + 1) * HT, :
            ],
        )
        return dst

    def cast_half(f_full, src32, hf):  # DVE
        nc.vector.tensor_copy(f_full[:, hf * HW : (hf + 1) * HW], src32[:])

    def fold_half(f_full, hf):  # sync xbar
        fold = foldp.tile([P, 4 * P], f16, tag="fold")
        nc.sync.dma_start_transpose(
            out=fold[:].rearrange("p (m r) -> p m r", r=P),
            in_=f_full[:, hf * HW : (hf + 1) * HW],
        )
        return fold

    def unfold_half(tall, fold, hf):  # gpsimd x2
        d4 = tall[0:DH, hf * PH : (hf + 1) * PH].rearrange(
            "d (m j r) -> d m j r", j=2, r=P
        )
        f3 = fold[:].rearrange("p (m r) -> p m r", r=P)
        nc.gpsimd.dma_start(out=d4[:, :, 0, :], in_=f3[0:DH])
        nc.gpsimd.dma_start(out=d4[:, :, 1, :], in_=f3[DH:P])

    def dup_half(tall, hf):  # sync: copy K^T/Q^T strip to partitions 64..127
        nc.sync.dma_start(
            out=tall[DH:P, hf * PH : (hf + 1) * PH],
            in_=tall[0:DH, hf * PH : (hf + 1) * PH],
        )

    def load_v_half(v3, h, hf):  # gpsimd SWDGE cast-DMA fp32->fp16
        nc.gpsimd.dma_start(
            out=v3[:, hf * HT : (hf + 1) * HT, 0:DH],
            in_=v_d[h].rearrange("(n p) d -> p n d", p=P)[
                :, hf * HT : (hf + 1) * HT, :
            ],
        )

    def dcomb_half(t, hf):  # DVE chain: dcomb strip for k-tiles of this half
        sl = slice(hf * HT, (hf + 1) * HT)
        t2_3 = t["t2"][:].rearrange("p (n d) -> p n d", d=DH)
        qf3 = t["qf"][:].rearrange("p (n d) -> p n d", d=DH)
        dcomb3 = t["dcomb"][:].rearrange("p (n j) -> p n j", j=P)
        nc.vector.tensor_mul(
            t2_3[:, sl, :], qf3[:, sl, :], rd16[:, None, :].to_broadcast([P, HT, DH])
        )
        nc.vector.tensor_reduce(
            out=t["vq"][:, sl], in_=t2_3[:, sl, :],
            axis=mybir.AxisListType.X, op=mybir.AluOpType.add,
        )
        nc.vector.tensor_copy(t["vq16"][:, sl], t["vq"][:, sl])
        nc.vector.tensor_mul(
            dcomb3[:, sl, :],
            idp[:, None, :].to_broadcast([P, HT, P]),
            t["vq16"][:, sl, None].to_broadcast([P, HT, P]),
        )
        nc.vector.tensor_add(
            dcomb3[:, sl, :], dcomb3[:, sl, :],
            bneg[:, None, :].to_broadcast([P, HT, P]),
        )

    # ---- main loop ------------------------------------------------------
    def emit_qk_group(group, t):
        gcol, items = group
        dcomb3 = t["dcomb"][:].rearrange("p (n j) -> p n j", j=P)
        chunks = []  # (psc col, q start, width, fill, is_first_chunk)
        for f, off in items:
            for so in range(0, f.n, 512):
                chunks.append(
                    (gcol + off + so, f.base + so, min(512, f.n - so), f, so == 0)
                )

        def emit_chunk(c, hi):
            col, qs, w, f, first = c
            rows = slice(DH, P) if hi else slice(0, DH)
            nc.tensor.matmul(
                psc[:, col : col + w],
                lhsT=t["ktall"][rows, f.q0 : f.q0 + P],
                rhs=t["qtall"][rows, qs : qs + w],
                start=True,
                stop=(not f.diag) if first else True,
                skip_group_check=True,
            )

        # chunks start at distinct 512-aligned cols -> distinct PSUM banks,
        # so consecutive chunks pack as concurrent lo/hi row-group matmuls
        i = 0
        while i < len(chunks):
            emit_chunk(chunks[i], hi=False)
            if i + 1 < len(chunks):
                emit_chunk(chunks[i + 1], hi=True)
                i += 2
            else:
                i += 1
        for f, off in items:
            if f.diag:
                nc.tensor.matmul(
                    psc[:, gcol + off : gcol + off + P],
                    lhsT=aui[:], rhs=dcomb3[:, f.ki, :],
                    start=False, stop=True, skip_group_check=True,
                )

    def emit_act(group):
        gcol, items = group
        gw = items[-1][1] + items[-1][0].n
        slab = slabp.tile([P, 2 * PH], f16, tag="slab")
        nc.scalar.activation(slab[:, 0:gw], psc[:, gcol : gcol + gw], AF.Exp,
                             scale=INV_SCALE)
        return slab

    def main_loop(h, t, slot_state):
        v3 = t["vaug"][:].rearrange("p (n e) -> p n e", e=VW)
        for ph in (1, 0):
            lo, hi = ph * PH, (ph + 1) * PH
            outT = pout.tile([DH + 1, PH], f32, tag="outT")
            seq = _assign_slots(_plan_phase(lo, hi), slot_state)
            # PV start/stop bookkeeping per 512-col block, in emission order
            order = {}
            first_touch, last_touch = {}, {}
            for idx, f in enumerate(ff for _, items in seq for ff, _ in items):
                order[id(f)] = idx
                for qb in range(f.base // 512, (f.base + f.n - 1) // 512 + 1):
                    first_touch.setdefault(qb, idx)
                    last_touch[qb] = idx

            def epilogue_chunk(qb):
                c0 = qb * 512
                nc.vector.tensor_copy(
                    t["outTs"][0 : DH + 1, c0 : c0 + 512],
                    outT[:, c0 - lo : c0 - lo + 512],
                )
                onat = outp.tile([P, 4 * OW], f16, tag="onat")
                onat3 = onat[:].rearrange("p (n e) -> p n e", e=OW)
                nc.sync.dma_start_transpose(
                    out=onat3, in_=t["outTs"][:, c0 : c0 + 512]
                )
                recip = outp.tile([P, 4], f32, tag="recip")
                nc.vector.reciprocal(recip[:, :, None], onat3[:, :, DH : DH + 1])
                ofin = outp.tile([P, 4 * DH], f32, tag="ofin")
                nc.vector.tensor_mul(
                    ofin[:].rearrange("p (n d) -> p n d", d=DH),
                    onat3[:, :, 0:DH],
                    recip[:, :, None].to_broadcast([P, 4, DH]),
                )
                nc.sync.dma_start(
                    out=out_d[h].rearrange("(n p) d -> p n d", p=P)[
                        :, qb * 4 : (qb + 1) * 4, :
                    ],
                    in_=ofin[:].rearrange("p (n d) -> p n d", d=DH),
                )

            def emit_pv_group(group, slab):
                gcol, items = group
                for f, off in items:
                    for qb in range(f.base // 512, (f.base + f.n - 1) // 512 + 1):
                        g0 = max(f.base, qb * 512)
                        g1 = min(f.base + f.n, qb * 512 + 512)
                        idx = order[id(f)]
                        nc.tensor.matmul(
                            outT[:, g0 - lo : g1 - lo],
                            lhsT=v3[:, f.ki, 0 : DH + 1],
                            rhs=slab[:, off + (g0 - f.base) : off + (g1 - f.base)],
                            start=(first_touch[qb] == idx),
                            stop=(last_touch[qb] == idx),
                            skip_group_check=True,
                        )
                        if last_touch[qb] == idx:
                            epilogue_chunk(qb)

            emit_qk_group(seq[0], t)
            if len(seq) > 1:
                emit_qk_group(seq[1], t)
            pend = []
            for gi, group in enumerate(seq):
                slab = emit_act(group)
                if gi + 2 < len(seq):
                    emit_qk_group(seq[gi + 2], t)
                if pend:
                    emit_pv_group(*pend.pop(0))
                pend.append((group, slab))
            while pend:
                emit_pv_group(*pend.pop(0))

    # ---- schedule -------------------------------------------------------
    # Engine-queue emission order is chosen so the phase-1 critical path
    # (k.a load -> cast -> fold -> unfold -> dup -> first QK pair, plus
    # q.b the same) is unobstructed; v loads and the .b/.a halves follow.
    t0 = head_tiles()
    t1 = head_tiles()

    k0a = load_half(k_d, 0, 0, "k32")   # sync
    q0b = load_half(q_d, 0, 1, "q32")
    cast_half(t0["kf"], k0a, 0)         # DVE
    cast_half(t0["qf"], q0b, 1)
    fk0a = fold_half(t0["kf"], 0)       # sync
    fq0b = fold_half(t0["qf"], 1)
    v30 = t0["vaug"][:].rearrange("p (n e) -> p n e", e=VW)
    v31 = t1["vaug"][:].rearrange("p (n e) -> p n e", e=VW)
    nc.gpsimd.memset(v30[:, :, DH : DH + 1], 1.0)  # gpsimd, before v loads
    unfold_half(t0["ktall"], fk0a, 0)   # gpsimd
    unfold_half(t0["qtall"], fq0b, 1)
    dup_half(t0["ktall"], 0)            # sync
    dup_half(t0["qtall"], 1)
    k0b = load_half(k_d, 0, 1, "k32")   # sync
    q0a = load_half(q_d, 0, 0, "q32")
    load_v_half(v30, 0, 0)              # gpsimd (cast-DMA)
    dcomb_half(t0, 1)                   # DVE (phase 1 diag fills)
    cast_half(t0["kf"], k0b, 1)         # DVE
    cast_half(t0["qf"], q0a, 0)
    fk0b = fold_half(t0["kf"], 1)       # sync
    fq0a = fold_half(t0["qf"], 0)
    unfold_half(t0["ktall"], fk0b, 1)   # gpsimd
    dup_half(t0["ktall"], 1)            # sync
    load_v_half(v30, 0, 1)              # gpsimd
    unfold_half(t0["qtall"], fq0a, 0)   # gpsimd
    dup_half(t0["qtall"], 0)            # sync
    dcomb_half(t0, 0)                   # DVE
    nc.gpsimd.memset(t0["outTs"][DH:OW, :], 0.0)

    # head 1 prep (emitted before head 0's main loop so its DVE/gpsimd/sync
    # work fills those queues while head 0's exp stream runs)
    k1a = load_half(k_d, 1, 0, "k32")
    q1b = load_half(q_d, 1, 1, "q32")
    cast_half(t1["kf"], k1a, 0)
    cast_half(t1["qf"], q1b, 1)
    fk1a = fold_half(t1["kf"], 0)
    fq1b = fold_half(t1["qf"], 1)
    nc.gpsimd.memset(v31[:, :, DH : DH + 1], 1.0)
    unfold_half(t1["ktall"], fk1a, 0)
    unfold_half(t1["qtall"], fq1b, 1)
    dup_half(t1["ktall"], 0)
    dup_half(t1["qtall"], 1)
    k1b = load_half(k_d, 1, 1, "k32")
    q1a = load_half(q_d, 1, 0, "q32")
    load_v_half(v31, 1, 0)
    dcomb_half(t1, 1)
    cast_half(t1["kf"], k1b, 1)
    cast_half(t1["qf"], q1a, 0)
    fk1b = fold_half(t1["kf"], 1)
    fq1a = fold_half(t1["qf"], 0)
    unfold_half(t1["ktall"], fk1b, 1)
    dup_half(t1["ktall"], 1)
    load_v_half(v31, 1, 1)
    unfold_half(t1["qtall"], fq1a, 0)
    dup_half(t1["qtall"], 0)
    dcomb_half(t1, 0)
    nc.gpsimd.memset(t1["outTs"][DH:OW, :], 0.0)

    slot_state = {"prev": set()}
    main_loop(0, t0, slot_state)
    main_loop(1, t1, slot_state)


def build_nc(debug=False):
    from contextlib import ExitStack

    nc = bacc.Bacc("TRN2", target_bir_lowering=False, debug=debug, num_devices=N_CORES)
    q_d = nc.dram_tensor("query", [HPC, S, DH], f32, kind="ExternalInput").ap()
    k_d = nc.dram_tensor("key", [HPC, S, DH], f32, kind="ExternalInput").ap()
    v_d = nc.dram_tensor("value", [HPC, S, DH], f32, kind="ExternalInput").ap()
    rw_d = nc.dram_tensor("R_w", [3, DH], f32, kind="ExternalInput").ap()
    rb_d = nc.dram_tensor("R_b", [3], f32, kind="ExternalInput").ap()
    out_d = nc.dram_tensor("out", [HPC, S, DH], f32, kind="ExternalOutput").ap()
    with tile.TileContext(nc) as tc, ExitStack() as ctx:
        _emit(ctx, tc, q_d, k_d, v_d, rw_d, rb_d, out_d)
    nc.finalize()
    return nc


_NC_CACHE = {}


def _get_nc():
    if "nc" not in _NC_CACHE:
        _NC_CACHE["nc"] = build_nc()
    return _NC_CACHE["nc"]


def kernel(query, key, value, R_w, R_b, trace=False):
    query = np.ascontiguousarray(np.asarray(query, dtype=np.float32))
    key = np.ascontiguousarray(np.asarray(key, dtype=np.float32))
    value = np.ascontiguousarray(np.asarray(value, dtype=np.float32))
    R_w = np.ascontiguousarray(np.asarray(R_w, dtype=np.float32))
    R_b = np.ascontiguousarray(np.asarray(R_b, dtype=np.float32))

    nc = _get_nc()
    in_maps = [
        {
            "query": query[c * HPC : (c + 1) * HPC],
            "key": key[c * HPC : (c + 1) * HPC],
            "value": value[c * HPC : (c + 1) * HPC],
            "R_w": R_w,
            "R_b": R_b,
        }
        for c in range(N_CORES)
    ]
    res = run_bass_kernel_spmd(nc, in_maps, core_ids=list(range(N_CORES)), trace=trace)
    out = np.concatenate([res.results[c]["out"] for c in range(N_CORES)], axis=0)
    if trace:
        kernel.last_results = res
    return out.astype(np.float32, copy=False)
